# revision 1
# baseline (speedup 1.0000x reference)
"""Trainium2 Bass kernel for a 3-layer complex RBF network.

Math per layer (complex y, G; real phi):
    dist_i = sum_j |y_j - G_ij|^2
    phi    = exp(-dist / (2 s))
    y_out  = W @ phi + b        (complex W, b)

Distribution (8 cores): shard the hidden axis I=4096 -> 512 rows of G / columns
of W per core.  dist/phi are computed fully locally per shard; the matvec
W[:, shard] @ phi_shard yields a full-length partial y that is AllReduce-summed
across cores (b is added as b/8 on every core's partial before the reduce).

Per-core implementation (NeuronCore engines):
  - G arrives natural-layout [128 i x Oprev] (2 i-chunks per DMA) via SWDGE
    cast-DMA (fp32->bf16); DVE computes diff = G - y_bcast in place; ACT
    computes Square(diff) with accum_out -> dist [128,1] chunks.
  - phi = ACT Exp(clamp(dist * (-1/(2s)))) lands as [128,1] chunks: the
    stationary (lhsT) layout the PE matvec needs.
  - W arrives as batched tiles [128 o x (4 ot x 512 i)] (SWDGE cast to bf16),
    then ONE xbar transpose per batch emits 16 transposed 128x128 blocks
    (block c2 = b*4+ic holds W^T[i-chunk ic, o-tile os*4+b]); the PE matvec
    streams rhs [128 i, (4 b x 128 o)] = N=512 per i-chunk, accumulating
    y_partial[1, 512] in PSUM.
  - AllReduce (gpsimd/ncfw) sums partial y; the y recycle for the next layer
    is a single DRAM->DRAM cast DMA (fp32->bf16) + partition-broadcast DMA.

Weights are read from HBM exactly once (fp32) -- the memory roofline.
Instruction-count discipline matters more than anything here: DMA/transpose
instructions cost ~0.6-1.2us of issuing-engine time and multi-sem waits
legalize into ~1-2us EventSemaphores, so everything is batched.

Engine rings: weight loads own the gpsimd (SWDGE) ring, ordered so the next
layer's stream sits before the current layer's AllReduce; y plumbing (ccin
store, broadcast) lives on the scalar HWDGE ring; xbar transposes own the
sync ring.
"""

import numpy as np

P = 128
NCORES = 8
HID = 4096
IS = HID // NCORES          # 512: per-core shard of the hidden axis
NCH = IS // P               # 4 chunks of 128
# (Oprev, Ol) for layers 1..3
DIMS = [(1024, 4096), (4096, 4096), (4096, 1024)]

_cache = {}


def _build_nc():
    import concourse.bacc as bacc
    import concourse.mybir as mybir
    import concourse.tile as tile

    f32 = mybir.dt.float32
    bf16 = mybir.dt.bfloat16
    AF = mybir.ActivationFunctionType
    ALU = mybir.AluOpType

    # Bacc (not raw Bass): its compile() pass legalizes multi-sem waits into
    # InstEventSemaphore carriers (HW instructions hold only 1 wait slot).
    nc = bacc.Bacc(None)

    x = nc.dram_tensor("x", [2, 1024], f32, kind="ExternalInput")
    W, G, S, B = {}, {}, {}, {}
    for l, (Op, Ol) in enumerate(DIMS, start=1):
        W[l] = nc.dram_tensor(f"W{l}s", [2, Ol, IS], f32, kind="ExternalInput")
        G[l] = nc.dram_tensor(f"G{l}s", [2, IS, Op], f32, kind="ExternalInput")
        S[l] = nc.dram_tensor(f"s{l}s", [IS], f32, kind="ExternalInput")
        B[l] = nc.dram_tensor(f"b{l}f", [2, Ol], f32, kind="ExternalInput")
    out = nc.dram_tensor("out", [2, 1024], f32, kind="ExternalOutput")

    with tile.TileContext(nc) as tc:
        with (
            tc.tile_pool(name="gnat", bufs=2) as gnat,      # [128, 2, Op] bf16
            tc.tile_pool(name="wnat", bufs=4) as wnatp,     # [128, 4, 512] bf16
            tc.tile_pool(name="wT", bufs=18) as wTp,        # [128, 16, 128] bf16
            tc.tile_pool(name="ybc", bufs=3) as ybcp,       # [128, Op] bf16
            tc.tile_pool(name="small", bufs=1) as small,
            tc.tile_pool(name="psum", bufs=4, space="PSUM") as psump,
            tc.tile_pool(name="dram", bufs=1, space="DRAM") as dramp,
        ):
            # ---------------- preloads ----------------------------------------
            # s -> -1/(2s) as [128, NCH] chunks; b -> b/8 staged in DRAM
            n2s, b8d = {}, {}
            for l, (Op, Ol) in enumerate(DIMS, start=1):
                s4 = small.tile([P, NCH], f32, tag=f"s4_{l}")
                nc.gpsimd.dma_start(s4[:], S[l][:].rearrange("(c p) -> p c", p=P))
                rec = small.tile([P, NCH], f32, tag=f"rec_{l}")
                nc.vector.reciprocal(rec[:], s4[:])
                t = small.tile([P, NCH], f32, tag=f"n2s_{l}")
                nc.vector.tensor_scalar_mul(t[:], rec[:], -0.5)
                n2s[l] = t

                bsc = small.tile([1, 2 * Ol], f32, tag="row32")  # shared scratch
                nc.scalar.dma_start(bsc[:], B[l][:])
                nc.vector.tensor_scalar_mul(bsc[:], bsc[:], 1.0 / NCORES)
                bd = dramp.tile([2, Ol], f32, tag=f"b8_{l}")
                nc.scalar.dma_start(bd[:], bsc[:])
                b8d[l] = bd

            # scratch used by "touch" ops that pre-absorb DMA-completion waits
            touch = small.tile([1, 2], bf16, tag="touch")

            # ---------------- y broadcast for layer 1 (from x, cast) ----------
            ybct = {}
            for r in range(2):
                yb = ybcp.tile([P, DIMS[0][0]], bf16, tag="ybc")
                nc.gpsimd.dma_start(yb[:], x[r : r + 1, :].partition_broadcast(P))
                nc.vector.tensor_copy(touch[:, r : r + 1], yb[0:1, 0:1])
                ybct[(1, r)] = yb

            # ---------------- weight-load emission helpers --------------------
            gt = {}    # (l, r, cp) -> [128, 2, Op] natural G tile (chunks 2cp, 2cp+1)
            wTt = {}   # (l, r, os) -> [128, 16, 128] transposed W batch

            def emit_g_loads(l):
                Op = DIMS[l - 1][0]
                for cp in range(NCH // 2):
                    for r in range(2):
                        g = gnat.tile([P, 2, Op], bf16, tag="gnat")
                        nc.gpsimd.dma_start(
                            g[:],
                            G[l][r, cp * 2 * P : (cp + 1) * 2 * P, :].rearrange(
                                "(c p) j -> p c j", p=P
                            ),
                        )
                        gt[(l, r, cp)] = g

            def emit_w_loads(l, ros):
                """Batched SWDGE cast-load + ONE xbar transpose per o-slice."""
                for r, os_ in ros:
                    wn = wnatp.tile([P, 4, 512], bf16, tag="wnat")
                    nc.gpsimd.dma_start(
                        wn[:],
                        W[l][r, os_ * 512 : (os_ + 1) * 512, :].rearrange(
                            "(b p) i -> p b i", p=P
                        ),
                    )
                    wt = wTp.tile([P, 16, P], bf16, tag="wT")
                    nc.sync.dma_start(wt[:], wn[:], transpose=True)
                    wTt[(l, r, os_)] = wt

            def w_ros(l, lo, hi):
                Ol = DIMS[l - 1][1]
                allr = [(r, os_) for r in range(2) for os_ in range(Ol // 512)]
                return allr[lo:hi]

            # Prefetch order on the gpsimd ring:
            # G1, W1, G2, W2[:6] | b1acc AR1 | W2[6:], ybf1, G3, W3[:2] | b2acc
            # AR2 | W3[2:], ybf2 | b3acc AR3 | out
            emit_g_loads(1)
            emit_w_loads(1, w_ros(1, 0, 10**9))
            emit_g_loads(2)
            emit_w_loads(2, w_ros(2, 0, 6))

            # ---------------- per-layer compute --------------------------------
            ysrc = None  # DRAM bf16 [2, Op] holding y of the previous layer
            for l, (Op, Ol) in enumerate(DIMS, start=1):
                NOS = Ol // 512

                if l > 1:
                    for r in range(2):
                        yb = ybcp.tile([P, Op], bf16, tag="ybc")
                        nc.scalar.dma_start(
                            yb[:], ysrc[r : r + 1, :].partition_broadcast(P)
                        )
                        nc.vector.tensor_copy(touch[:, r : r + 1], yb[0:1, 0:1])
                        ybct[(l, r)] = yb

                # ---- dist: g = G - y_bcast (DVE, in place), Square+accum (ACT)
                dacc = small.tile([P, 2 * NCH], f32, tag=f"dacc_{l}")
                for cp in range(NCH // 2):
                    for r in range(2):
                        g = gt[(l, r, cp)]
                        for ci in range(2):
                            c = 2 * cp + ci
                            gs = g[:, ci, :]
                            nc.vector.tensor_sub(gs, gs, ybct[(l, r)][:])
                            nc.scalar.activation(
                                gs, gs, AF.Square,
                                accum_out=dacc[:, 2 * c + r : 2 * c + r + 1],
                            )

                # ---- phi = exp(clamp((d_re+d_im) * -1/(2s), -85)) ----
                phi = small.tile([P, NCH], bf16, tag=f"phi_{l}")
                expin = small.tile([P, NCH], f32, tag=f"expin_{l}")
                junk2 = small.tile([P, 2], f32, tag=f"junk_{l}")
                for c in range(NCH):
                    nc.vector.tensor_scalar(
                        junk2[:], dacc[:, 2 * c : 2 * c + 2],
                        n2s[l][:, c : c + 1], 0.0, ALU.mult, ALU.add,
                        accum_out=expin[:, c : c + 1],
                    )
                    nc.vector.tensor_scalar_max(
                        expin[:, c : c + 1], expin[:, c : c + 1], -85.0
                    )
                    nc.scalar.activation(
                        phi[:, c : c + 1], expin[:, c : c + 1], AF.Exp
                    )

                # ---- y_partial = W_shard @ phi (PE), PSUM -> SBUF (ACT) ----
                # single-row layout [1, 2*Ol]: row = comp0 ++ comp1
                ysb = small.tile([1, 2 * Ol], f32, tag="row32")
                for r in range(2):
                    for os_ in range(NOS):
                        wt = wTt[(l, r, os_)]
                        # rhs for i-chunk ic: blocks c2 = b*4+ic, b=0..3
                        w4 = wt[:].rearrange("p (b ic) f -> p ic b f", ic=NCH)
                        ps = psump.tile([1, 512], f32, tag="psy")
                        for ic in range(NCH):
                            nc.tensor.matmul(
                                ps[:],
                                phi[:, ic : ic + 1],
                                w4[:, ic, :, :],
                                start=(ic == 0),
                                stop=(ic == NCH - 1),
                            )
                        off = r * Ol + os_ * 512
                        # scalar engine: merges the matmuls' phi-wait and the
                        # psum-slot WAR wait onto one engine clock
                        nc.scalar.copy(ysb[:, off : off + 512], ps[:])

                # ---- partial y + b/8, then AllReduce across the 8 cores ----
                ccin = dramp.tile([2, Ol], f32, tag=f"ccin_{l}")
                ccout = dramp.tile([2, Ol], f32, tag=f"ccout_{l}")
                nc.scalar.dma_start(ccin[:], ysb[:])
                nc.gpsimd.dma_start(ccin[:], b8d[l][:], accum_op=ALU.add)
                nc.gpsimd.collective_compute(
                    "AllReduce",
                    ALU.add,
                    replica_groups=[list(range(NCORES))],
                    ins=[ccin.opt()],
                    outs=[ccout.opt()],
                )
                # keep next-layer weight prefetch flowing on the gpsimd ring;
                # the DRAM->DRAM y cast must come before anything that
                # transitively feeds the layer after next (deadlock audit in
                # the module docstring notes)
                if l == 1:
                    emit_w_loads(2, w_ros(2, 6, 10**9))
                    ybf = dramp.tile([2, Ol], bf16, tag=f"ybf_{l}")
                    nc.gpsimd.dma_start(ybf[:], ccout[:])  # cast f32->bf16
                    ysrc = ybf
                    emit_g_loads(3)
                    emit_w_loads(3, w_ros(3, 0, 2))
                elif l == 2:
                    emit_w_loads(3, w_ros(3, 2, 10**9))
                    ybf = dramp.tile([2, Ol], bf16, tag=f"ybf_{l}")
                    nc.gpsimd.dma_start(ybf[:], ccout[:])  # cast f32->bf16
                    ysrc = ybf
                else:
                    nc.gpsimd.dma_start(out[:], ccout[:])

    # Bacc.finalize runs compile(): reg alloc + event-semaphore legalization
    nc.finalize()
    return nc


def _get_nc():
    if "nc" not in _cache:
        _cache["nc"] = _build_nc()
    return _cache["nc"]


def make_in_maps(inputs):
    """Host-side sharding: slice the hidden axis into 8 shards."""
    in_maps = []
    for c in range(NCORES):
        lo, hi = c * IS, (c + 1) * IS
        m = {"x": np.ascontiguousarray(inputs["x"], dtype=np.float32)}
        for l in range(1, 4):
            m[f"W{l}s"] = np.ascontiguousarray(inputs[f"W{l}"][:, :, lo:hi], dtype=np.float32)
            m[f"G{l}s"] = np.ascontiguousarray(inputs[f"G{l}"][:, lo:hi, :], dtype=np.float32)
            m[f"s{l}s"] = np.ascontiguousarray(inputs[f"s{l}"][lo:hi], dtype=np.float32)
            m[f"b{l}f"] = np.ascontiguousarray(inputs[f"b{l}"], dtype=np.float32)
        in_maps.append(m)
    return in_maps


def run(inputs, trace=False, **kw):
    from concourse.bass_utils import run_bass_kernel_spmd

    nc = _get_nc()
    in_maps = make_in_maps(inputs)
    res = run_bass_kernel_spmd(nc, in_maps, list(range(NCORES)), trace=trace, **kw)
    return res


def kernel(**inputs):
    res = run(inputs, trace=False)
    return np.asarray(res.results[0]["out"], dtype=np.float32)



# revision 2
# speedup vs baseline: 1.2011x; 1.2011x over previous
"""Trainium2 Bass kernel v2 for the 3-layer complex RBF network.

Math per layer (complex y, G; real phi):
    dist_i = sum_j |y_j - G_ij|^2
           = ||y||^2 + ||G_i||^2 - 2 Re<G_i, y>
    phi    = exp(-dist / (2 s))
    y_out  = W @ phi + b        (complex W, b)

Distribution (8 cores): shard the hidden axis I=4096 -> 512 rows of G /
columns of W per core.  dist/phi are local per shard; the column-sharded
matvec partials are AllReduce-summed (b/8 folded into each core's partial
via the reduce seed).

v2 design vs the v1 (xbar-transpose + PE matvec) baseline:
  - Host pre-casts W/G/x to bf16 and pre-relayouts W/G so that every
    weight DMA is a plain HWDGE (sync-ring) copy with 8-16KB contiguous
    descriptors per partition: W as [2, 128 p, Ol/128 c, 512 i] (row
    o = c*128+p), G as [2, 128 p, 4 c, Oprev] (row i = c*128+p).
    Halves HBM traffic (36MB/core) and kills the SWDGE cast stream.
  - No xbar transposes at all: the matvec y_o = sum_i W[o,i] phi[i] runs
    on DVE as tensor_tensor_reduce(W_tile[128 o, 512 i] * phi_bcast) with
    free-axis accumulation; seed = b[o]/8 (host-prepped column layout).
  - dist via the expanded form: ||G_i||^2 (ACT Square+accum, runs as G
    tiles arrive, off the critical path), cross term -2Re<G,y> as two
    chained tensor_tensor_reduce ops against the y broadcast, ||y||^2
    likewise (seeded chain).  Post-AllReduce critical path per layer is
    just ybc DMA + ~10us of DVE + phi plumbing.
  - phi broadcast across partitions via PE: transpose phi [128,4] ->
    [4,128] (identity matmul), then 4 one-hot matmuls replicate each
    chunk row into PSUM [128, 512]; ACT casts to bf16 SBUF.
  - AllReduce in bf16 (16KB / 4KB), partial y transposed to row layout
    by one PE identity matmul; final output cast bf16->fp32 by one SWDGE
    DMA.

HBM per core: 36MB bf16 weights + ~3MB y broadcasts ~= 110us of DMA at
the ~340GB/s streaming rate; critical path (3 ARs + per-layer tails) puts
the target at ~150us.
"""

import numpy as np

P = 128
NCORES = 8
HID = 4096
IS = HID // NCORES          # 512: per-core shard of the hidden axis
NCH = IS // P               # 4 chunks of 128 hidden neurons
# (Oprev, Ol) for layers 1..3
DIMS = [(1024, 4096), (4096, 4096), (4096, 1024)]

_cache = {}


def _build_nc():
    import concourse.bacc as bacc
    import concourse.mybir as mybir
    import concourse.tile as tile
    from concourse import masks

    f32 = mybir.dt.float32
    bf16 = mybir.dt.bfloat16
    AF = mybir.ActivationFunctionType
    ALU = mybir.AluOpType

    nc = bacc.Bacc(None)

    x = nc.dram_tensor("x", [2, 1024], bf16, kind="ExternalInput")
    OH = nc.dram_tensor("oh", [NCH, NCH, P], bf16, kind="ExternalInput")
    W, G, N2S, B8 = {}, {}, {}, {}
    for l, (Op, Ol) in enumerate(DIMS, start=1):
        W[l] = nc.dram_tensor(f"W{l}", [2, P, Ol // P, IS], bf16, kind="ExternalInput")
        G[l] = nc.dram_tensor(f"G{l}", [2, P, NCH, Op], bf16, kind="ExternalInput")
        N2S[l] = nc.dram_tensor(f"n{l}", [P, NCH], f32, kind="ExternalInput")
        B8[l] = nc.dram_tensor(f"c{l}", [P, 2, Ol // P], f32, kind="ExternalInput")
    out = nc.dram_tensor("out", [2, 1024], f32, kind="ExternalOutput")

    with tile.TileContext(nc) as tc:
        with (
            tc.tile_pool(name="gpool", bufs=5) as gpool,    # G tiles
            tc.tile_pool(name="wpool", bufs=6) as wpool,    # W tiles
            tc.tile_pool(name="ybcp", bufs=4) as ybcp,      # y broadcast
            tc.tile_pool(name="small", bufs=1) as small,
            tc.tile_pool(name="psum", bufs=2, space="PSUM") as psump,
            tc.tile_pool(name="dram", bufs=1, space="DRAM") as dramp,
        ):
            # ---------------- preloads ----------------------------------
            ident = small.tile([P, P], bf16, tag="ident")
            masks.make_identity(nc, ident[:])
            onehot = []
            for c in range(NCH):
                oh = small.tile([NCH, P], bf16, tag=f"oh{c}")
                nc.scalar.dma_start(oh[:], OH[c, :, :])
                onehot.append(oh)
            n2s, b8 = {}, {}
            for l, (Op, Ol) in enumerate(DIMS, start=1):
                t = small.tile([P, NCH], f32, tag=f"n2s_{l}")
                nc.scalar.dma_start(t[:], N2S[l][:])
                n2s[l] = t
                t = small.tile([P, 2, Ol // P], f32, tag=f"b8_{l}")
                nc.scalar.dma_start(t[:], B8[l][:])
                b8[l] = t
            # dummy sink for DVE reduce ops (stride-0 out, qr.py-style)
            dumv = small.tile([P, 1], bf16, tag="dumv")
            # real junk sink for ACT Square elementwise output
            junka = small.tile([P, 4096], bf16, tag="junka")

            # ---------------- y broadcast for layer 1 (x, bf16) ---------
            ybct = {}
            for r in range(2):
                yb = ybcp.tile([P, DIMS[0][0]], bf16, tag="ybc")
                nc.scalar.dma_start(yb[:], x[r : r + 1, :].partition_broadcast(P))
                ybct[(1, r)] = yb

            # ---------------- weight stream (sync ring, HWDGE) ----------
            # Program order on the sync ring == consumption order:
            # G1 W1 G2 W2 G3 W3.  Pools provide the AR-stall ride-through.
            gt = {}   # (l, r, cp) -> G tile; L1: [128,4,1024] cp=0; L2/3: [128,2,Op]
            wt = {}   # (l, r, k)  -> W tile [128, 8, 512]
            for l, (Op, Ol) in enumerate(DIMS, start=1):
                if l == 1:
                    for r in range(2):
                        g = gpool.tile([P, NCH, Op], bf16, tag="g")
                        nc.sync.dma_start(g[:], G[l][r, :, :, :])
                        gt[(l, r, 0)] = g
                else:
                    for cp in range(NCH // 2):
                        for r in range(2):
                            g = gpool.tile([P, 2, Op], bf16, tag="g")
                            nc.sync.dma_start(g[:], G[l][r, :, 2 * cp : 2 * cp + 2, :])
                            gt[(l, r, cp)] = g
                for r in range(2):
                    for k in range(Ol // P // 8):
                        w = wpool.tile([P, 8, IS], bf16, tag="w")
                        nc.sync.dma_start(w[:], W[l][r, :, 8 * k : 8 * k + 8, :])
                        wt[(l, r, k)] = w

            def gslice(l, r, c):
                """[128, Op] AP for hidden-chunk c, component r of layer l."""
                if l == 1:
                    return gt[(l, r, 0)][:, c, :]
                return gt[(l, r, c // 2)][:, c % 2, :]

            # ---------------- per-layer compute --------------------------
            ysrc = None  # previous layer's AR output (DRAM bf16 [2, Op])
            for l, (Op, Ol) in enumerate(DIMS, start=1):
                NOC = Ol // P  # number of 128-wide output chunks

                if l > 1:
                    for r in range(2):
                        yb = ybcp.tile([P, Op], bf16, tag="ybc")
                        nc.scalar.dma_start(
                            yb[:], ysrc[r : r + 1, :].partition_broadcast(P)
                        )
                        ybct[(l, r)] = yb

                # ---- ||G_i||^2: ACT Square + accum as tiles arrive ------
                # gg col layout: 4*r + c
                gg = small.tile([P, 2 * NCH], f32, tag=f"gg_{l}")
                for r in range(2):
                    for c in range(NCH):
                        nc.scalar.activation(
                            junka[:, :Op], gslice(l, r, c), AF.Square,
                            accum_out=gg[:, 4 * r + c : 4 * r + c + 1],
                        )
                ggs = small.tile([P, NCH], f32, tag=f"ggs_{l}")
                nc.vector.tensor_add(ggs[:], gg[:, 0:NCH], gg[:, NCH : 2 * NCH])

                # ---- cross term -2*sum_j G_r*y_r + ||y||^2 (DVE) --------
                cr0 = small.tile([P, NCH], f32, tag=f"cr0_{l}")
                cr1 = small.tile([P, NCH], f32, tag=f"cr1_{l}")
                yyA = small.tile([P, 1], f32, tag=f"yyA_{l}")
                yyB = small.tile([P, 1], f32, tag=f"yyB_{l}")
                for c in range(NCH):
                    nc.vector.scalar_tensor_tensor(
                        dumv[:].broadcast_to((P, Op)),
                        gslice(l, 0, c), -2.0, ybct[(l, 0)][:],
                        op0=ALU.mult, op1=ALU.mult,
                        accum_out=cr0[:, c : c + 1],
                    )
                nc.vector.scalar_tensor_tensor(
                    dumv[:].broadcast_to((P, Op)),
                    ybct[(l, 0)][:], 1.0, ybct[(l, 0)][:],
                    op0=ALU.mult, op1=ALU.mult,
                    accum_out=yyA[:],
                )
                for c in range(NCH):
                    nc.vector.scalar_tensor_tensor(
                        dumv[:].broadcast_to((P, Op)),
                        gslice(l, 1, c), -2.0, ybct[(l, 1)][:],
                        op0=ALU.mult, op1=ALU.mult,
                        accum_out=cr1[:, c : c + 1],
                    )
                nc.vector.scalar_tensor_tensor(
                    dumv[:].broadcast_to((P, Op)),
                    ybct[(l, 1)][:], 1.0, ybct[(l, 1)][:],
                    op0=ALU.mult, op1=ALU.mult,
                    accum_out=yyB[:],
                )

                # ---- phi = exp(max((cr0+cr1+ggs+yy) * -1/(2s), -85)) ----
                dist = small.tile([P, NCH], f32, tag=f"dist_{l}")
                expin = small.tile([P, NCH], f32, tag=f"expin_{l}")
                nc.vector.tensor_add(dist[:], cr0[:], cr1[:])
                nc.vector.tensor_add(dist[:], dist[:], ggs[:])
                nc.vector.tensor_scalar(
                    dist[:], dist[:], yyA[:], yyB[:], ALU.add, ALU.add
                )
                nc.vector.scalar_tensor_tensor(
                    expin[:], dist[:], 1.0, n2s[l][:],
                    op0=ALU.mult, op1=ALU.mult,
                )
                nc.vector.tensor_scalar_max(expin[:], expin[:], -85.0)
                phi = small.tile([P, NCH], bf16, tag=f"phi_{l}")
                nc.scalar.activation(phi[:], expin[:], AF.Exp)

                # ---- phi broadcast: [128,4] -> [4,128] -> [128,512] -----
                phiT = psump.tile([NCH, P], bf16, tag="phiT")
                nc.tensor.transpose(phiT[:], phi[:], ident[:])
                phi_ts = small.tile([NCH, P], bf16, tag=f"phits_{l}")
                nc.scalar.copy(phi_ts[:], phiT[:])
                phibc = psump.tile([P, IS], f32, tag="phibc")
                for c in range(NCH):
                    nc.tensor.matmul(
                        phibc[:, c * P : (c + 1) * P],
                        onehot[c][:], phi_ts[:],
                        start=True, stop=True,
                    )
                phi_bb = small.tile([P, IS], bf16, tag=f"phibb_{l}")
                nc.scalar.copy(phi_bb[:], phibc[:])

                # ---- y_partial[o] = sum_i W[o,i] phi[i] + b[o]/8 (DVE) --
                ycol = small.tile([P, 2, NOC], f32, tag=f"ycol_{l}")
                for r in range(2):
                    for k in range(NOC // 8):
                        w = wt[(l, r, k)]
                        for cc in range(8):
                            cg = 8 * k + cc
                            nc.vector.scalar_tensor_tensor(
                                dumv[:].broadcast_to((P, IS)),
                                w[:, cc, :], 1.0, phi_bb[:],
                                op0=ALU.mult, op1=ALU.mult,
                                accum_out=ycol[:, r, cg : cg + 1],
                            )
                nc.vector.tensor_add(ycol[:], ycol[:], b8[l][:])

                # ---- column -> row layout, store, AllReduce -------------
                ycb = small.tile([P, 2, NOC], bf16, tag=f"ycb_{l}")
                nc.scalar.copy(ycb[:], ycol[:])
                yT = psump.tile([2 * NOC, P], bf16, tag="yT")
                nc.tensor.transpose(
                    yT[:], ycb[:].rearrange("p r c -> p (r c)"), ident[:]
                )
                ccsb = small.tile([2 * NOC, P], f32, tag=f"ccsb_{l}")
                nc.scalar.copy(ccsb[:], yT[:])
                ccin = dramp.tile([2, Ol], f32, tag=f"ccin_{l}")
                ccout = dramp.tile([2, Ol], f32, tag=f"ccout_{l}")
                nc.scalar.dma_start(
                    ccin[:].rearrange("r (c p) -> (r c) p", p=P), ccsb[:]
                )
                nc.gpsimd.collective_compute(
                    "AllReduce",
                    ALU.add,
                    replica_groups=[list(range(NCORES))],
                    ins=[ccin.opt()],
                    outs=[ccout.opt()],
                )
                if l < 3:
                    # bf16 copy for the next layer's partition broadcast
                    ybf = dramp.tile([2, Ol], bf16, tag=f"ybf_{l}")
                    nc.gpsimd.dma_start(ybf[:], ccout[:])  # cast f32->bf16
                    ysrc = ybf
                else:
                    ysrc = ccout

            # final output (already fp32)
            nc.gpsimd.dma_start(out[:], ysrc[:])

    nc.finalize()
    return nc


def _get_nc():
    if "nc" not in _cache:
        _cache["nc"] = _build_nc()
    return _cache["nc"]


def make_in_maps(inputs):
    """Host-side sharding + bf16 cast + DMA-friendly relayouts."""
    import ml_dtypes

    bf16 = ml_dtypes.bfloat16
    x = np.ascontiguousarray(inputs["x"]).astype(bf16)
    oh = np.ascontiguousarray(
        np.broadcast_to(np.eye(NCH, dtype=np.float32)[:, :, None], (NCH, NCH, P))
    ).astype(bf16)
    base = {"x": x, "oh": oh}
    in_maps = []
    for cidx in range(NCORES):
        lo, hi = cidx * IS, (cidx + 1) * IS
        m = dict(base)
        for l, (Op, Ol) in enumerate(DIMS, start=1):
            Ws = np.asarray(inputs[f"W{l}"])[:, :, lo:hi]        # [2, Ol, IS]
            m[f"W{l}"] = np.ascontiguousarray(
                Ws.reshape(2, Ol // P, P, IS).transpose(0, 2, 1, 3)
            ).astype(bf16)
            Gs = np.asarray(inputs[f"G{l}"])[:, lo:hi, :]        # [2, IS, Op]
            m[f"G{l}"] = np.ascontiguousarray(
                Gs.reshape(2, NCH, P, Op).transpose(0, 2, 1, 3)
            ).astype(bf16)
            s = np.asarray(inputs[f"s{l}"])[lo:hi].astype(np.float64)
            m[f"n{l}"] = np.ascontiguousarray(
                (-0.5 / s).reshape(NCH, P).T
            ).astype(np.float32)
            b = np.asarray(inputs[f"b{l}"]).astype(np.float64) / NCORES
            m[f"c{l}"] = np.ascontiguousarray(
                b.reshape(2, Ol // P, P).transpose(2, 0, 1)
            ).astype(np.float32)
        in_maps.append(m)
    return in_maps


def run(inputs, trace=False, **kw):
    from concourse.bass_utils import run_bass_kernel_spmd

    nc = _get_nc()
    in_maps = make_in_maps(inputs)
    res = run_bass_kernel_spmd(nc, in_maps, list(range(NCORES)), trace=trace, **kw)
    return res


def kernel(**inputs):
    res = run(inputs, trace=False)
    return np.asarray(res.results[0]["out"], dtype=np.float32)
